# revision 5
# baseline (speedup 1.0000x reference)
"""Multi-head causal attention (B=2, S=2048, D=1024, H=16, hd=64) on 8 trn2
NeuronCores.

Sharding: core c -> batch b=c//4, head-group g=c%4 (4 heads = 256 contiguous
model dims). Each core computes q/k/v projections for its head group from the
full (transposed) batch-b input, runs causal attention for its 4 heads, and
applies its slice of the output projection, producing a partial [2048, 1024]
output. The host sums the 4 partials per batch.

Per-core kernel layout (all fp32 storage, fp32r matmuls):
  xT   [1024, 2048]  x[b].T              (d on partition axis)
  wqT/wkT/wvT [1024, 256]                w[o_slice, :].T
  woT  [256, 1024]                       wo[:, o_slice].T
  cmask8 [128, 128]                      8 * mask[0,0,:128,:128].T (diag block)
  y    [2048, 1024]  partial output

Scores are computed transposed (S^T[j,i] = k^T.T @ q^T) so the softmax
denominator comes free from the AV matmul via a ones-column appended to V,
and no transposes are needed anywhere. Causality is structural: only j<=i
blocks are computed; the 128x128 diagonal blocks get the (scaled) mask added.
exp() skips max-subtraction (scores are ~N(0,1); |S|<~7 so fp32 exp is safe,
matching the reference's softmax up to rounding).
"""

import sys

for p in ("/opt/trn_rl_repo", "/root/.axon_site/_ro/trn_rl_repo"):
    if p not in sys.path:
        sys.path.insert(0, p)

import numpy as np

B, S, DIM, H, HD = 2, 2048, 1024, 16, 64
NCORES = 8
HG = 4  # heads per core
OG = HG * HD  # 256 output dims per core
NB = S // 512  # 4 i-blocks of 512
NJ = S // 128  # 16 j-tiles of 128

_CACHE = {}


def _build():
    import concourse.tile as tile
    from concourse import bacc, mybir

    f32 = mybir.dt.float32
    f32r = mybir.dt.float32r
    Exp = mybir.ActivationFunctionType.Exp

    def r(ap):
        return ap.bitcast(f32r)

    nc = bacc.Bacc("TRN2", target_bir_lowering=False, debug=False, num_devices=NCORES)

    xT = nc.dram_tensor("xT", [DIM, S], f32r, kind="ExternalInput")
    wqT = nc.dram_tensor("wqT", [DIM, OG], f32r, kind="ExternalInput")
    wkT = nc.dram_tensor("wkT", [DIM, OG], f32r, kind="ExternalInput")
    wvT = nc.dram_tensor("wvT", [DIM, OG], f32r, kind="ExternalInput")
    woT = nc.dram_tensor("woT", [OG, DIM], f32r, kind="ExternalInput")
    cmask8 = nc.dram_tensor("cmask8", [128, 128], f32, kind="ExternalInput")
    onesd = nc.dram_tensor("onesd", [128, 64], f32r, kind="ExternalInput")
    y = nc.dram_tensor("y", [S, DIM], f32, kind="ExternalOutput")

    xT_r = xT.ap().rearrange("(t p) s -> t p s", p=128)  # [8,128,2048]
    wqT_r = wqT.ap().rearrange("(t p) o -> t p o", p=128)  # [8,128,256]
    wkT_r = wkT.ap().rearrange("(t p) o -> t p o", p=128)
    wvT_r = wvT.ap().rearrange("(t p) o -> t p o", p=128)
    woT_r = woT.ap().rearrange("(t p) e -> t p e", p=128)  # [2,128,1024]
    y_r = y.ap().rearrange("(t p) e -> t p e", p=128)  # [16,128,1024]

    with tile.TileContext(nc) as tc:
        with (
            tc.tile_pool(name="persist", bufs=1) as pp,
            tc.tile_pool(name="work", bufs=4) as wp,
            tc.tile_pool(name="psum", bufs=4, space="PSUM") as ps,
        ):
            # ---- persistent SBUF tiles -------------------------------------
            xt = [pp.tile([128, S], f32r, tag=f"xt{i}", name=f"xt{i}") for i in range(8)]
            wqt = [pp.tile([128, OG], f32r, tag=f"wq{i}", name=f"wq{i}") for i in range(8)]
            wkt = [pp.tile([128, OG], f32r, tag=f"wk{i}", name=f"wk{i}") for i in range(8)]
            wvt = [pp.tile([128, OG], f32r, tag=f"wv{i}", name=f"wv{i}") for i in range(8)]
            wot = [pp.tile([128, DIM], f32r, tag=f"wo{i}", name=f"wo{i}") for i in range(2)]
            cm = pp.tile([128, 128], f32, tag="cm")
            qT = [pp.tile([128, S], f32r, tag=f"qT{i}", name=f"qT{i}") for i in range(2)]
            kT = [pp.tile([128, S], f32r, tag=f"kT{i}", name=f"kT{i}") for i in range(2)]
            vv = [pp.tile([128, HG, HD + 1], f32r, tag=f"vv{i}", name=f"vv{i}") for i in range(NJ)]
            zT = [pp.tile([128, S], f32r, tag=f"zT{i}", name=f"zT{i}") for i in range(2)]
            ones1 = pp.tile([1, 64], f32r, tag="ones1")

            for i in range(8):
                nc.sync.dma_start(out=xt[i], in_=xT_r[i])
                nc.sync.dma_start(out=wkt[i], in_=wkT_r[i])
                nc.sync.dma_start(out=wqt[i], in_=wqT_r[i])
                nc.sync.dma_start(out=wvt[i], in_=wvT_r[i])
            for i in range(2):
                nc.sync.dma_start(out=wot[i], in_=woT_r[i])
            nc.sync.dma_start(out=cm, in_=cmask8.ap())
            nc.sync.dma_start(out=ones1, in_=onesd.ap()[0:1, :])

            # ---- q/k/v projections ----------------------------------------
            # qT/kT: [o 128-chunk, s 512-block] accumulated over 8 e-tiles.
            for m in range(2):
                for n in range(NB):
                    for dst, wt in ((kT, wkt), (qT, wqt)):
                        acc = ps.tile([128, 512], f32, tag="s")
                        for e in range(8):
                            nc.tensor.matmul(
                                acc,
                                r(wt[e][:, m * 128 : (m + 1) * 128]),
                                r(xt[e][:, n * 512 : (n + 1) * 512]),
                                start=(e == 0),
                                stop=(e == 7),
                            )
                        nc.scalar.copy(dst[m][:, n * 512 : (n + 1) * 512], acc)
            # v: natural layout [s 128-chunk, o], with a ones column per head.
            for s in range(NJ):
                acc = ps.tile([128, 512], f32, tag="s")
                for e in range(8):
                    nc.tensor.matmul(
                        acc[:, 0:OG],
                        r(xt[e][:, s * 128 : (s + 1) * 128]),
                        r(wvt[e]),
                        start=(e == 0),
                        stop=(e == 7),
                    )
                nc.vector.tensor_copy(
                    vv[s][:, :, 0:HD],
                    acc[:, 0:OG].rearrange("p (h d) -> p h d", h=HG),
                )
                nc.sync.dma_start(
                    out=vv[s][:, :, HD : HD + 1],
                    in_=onesd.ap()[:, 0:HG].rearrange("p (h o) -> p h o", o=1),
                )

            # ---- attention per head ---------------------------------------
            for h in range(HG):
                m, po = divmod(h, 2)
                po *= 64
                for ib in range(NB):
                    psz = ps.tile([65, 512], f32, tag="z", bufs=2)
                    nplain = 4 * ib
                    njb = nplain + 4
                    for jb in range(njb):
                        last = jb == njb - 1
                        if jb < nplain:  # full (unmasked) block
                            pss = ps.tile([128, 512], f32, tag="s")
                            nc.tensor.matmul(
                                pss,
                                r(kT[m][po : po + 64, jb * 128 : (jb + 1) * 128]),
                                r(qT[m][po : po + 64, ib * 512 : (ib + 1) * 512]),
                                start=True,
                                stop=True,
                            )
                            ex = wp.tile([128, 512], f32r, tag="ex")
                            nc.scalar.activation(ex, pss, Exp, scale=0.125)
                            nc.tensor.matmul(
                                psz,
                                r(vv[jb][:, h, :]),
                                r(ex),
                                start=(jb == 0),
                                stop=last,
                            )
                        else:  # diagonal band block
                            t = jb - nplain
                            off = 128 * t
                            pss = ps.tile([128, 512], f32, tag="s")
                            nc.tensor.matmul(
                                pss[:, off:512],
                                r(kT[m][po : po + 64, jb * 128 : (jb + 1) * 128]),
                                r(qT[m][po : po + 64, ib * 512 + off : (ib + 1) * 512]),
                                start=True,
                                stop=True,
                            )
                            ex = wp.tile([128, 512], f32r, tag="ex")
                            sd = wp.tile([128, 128], f32, tag="sd", bufs=3)
                            nc.vector.tensor_add(sd, pss[:, off : off + 128], cm)
                            nc.scalar.activation(
                                ex[:, off : off + 128], sd, Exp, scale=0.125
                            )
                            if t < 3:
                                nc.scalar.activation(
                                    ex[:, off + 128 : 512],
                                    pss[:, off + 128 : 512],
                                    Exp,
                                    scale=0.125,
                                )
                            nc.tensor.matmul(
                                psz[:, off:512],
                                r(vv[jb][:, h, :]),
                                r(ex[:, off:512]),
                                start=(jb == 0),
                                stop=last,
                            )
                    # normalize: psz row 64 holds the softmax denominator.
                    dn = wp.tile([1, 512], f32r, tag="dn", bufs=2)
                    nc.vector.tensor_copy(dn, psz[64:65, :])
                    psb = ps.tile([128, 512], f32, tag="s")
                    nc.tensor.matmul(
                        psb[0:64, :], r(ones1), r(dn), start=True, stop=True
                    )
                    rc = wp.tile([64, 512], f32, tag="rc", bufs=2)
                    nc.vector.reciprocal(rc, psb[0:64, :])
                    nc.vector.tensor_mul(
                        zT[m][po : po + 64, ib * 512 : (ib + 1) * 512],
                        psz[0:64, :],
                        rc,
                    )

            # ---- output projection ----------------------------------------
            for s in range(NJ):
                ysb = wp.tile([128, DIM], f32, tag="ysb", bufs=3, name="ysb")
                for n2 in range(2):
                    psy = ps.tile([128, 512], f32, tag="s")
                    for kk in range(2):
                        nc.tensor.matmul(
                            psy,
                            r(zT[kk][:, s * 128 : (s + 1) * 128]),
                            r(wot[kk][:, n2 * 512 : (n2 + 1) * 512]),
                            start=(kk == 0),
                            stop=(kk == 1),
                        )
                    nc.scalar.copy(ysb[:, n2 * 512 : (n2 + 1) * 512], psy)
                nc.sync.dma_start(out=y_r[s], in_=ysb)

    nc.compile()
    return nc


def _get_nc():
    if "nc" not in _CACHE:
        _CACHE["nc"] = _build()
    return _CACHE["nc"]


def _in_maps(x, mask, wq, wk, wv, wo):
    cm8 = np.ascontiguousarray(8.0 * np.asarray(mask)[0, 0, :128, :128].T, np.float32)
    maps = []
    for c in range(NCORES):
        b, g = divmod(c, HG)
        sl = slice(OG * g, OG * (g + 1))
        maps.append(
            {
                "xT": np.ascontiguousarray(np.asarray(x)[b].T, np.float32),
                "wqT": np.ascontiguousarray(np.asarray(wq)[sl, :].T, np.float32),
                "wkT": np.ascontiguousarray(np.asarray(wk)[sl, :].T, np.float32),
                "wvT": np.ascontiguousarray(np.asarray(wv)[sl, :].T, np.float32),
                "woT": np.ascontiguousarray(np.asarray(wo)[:, sl].T, np.float32),
                "cmask8": cm8,
                "onesd": np.ones((128, 64), np.float32),
            }
        )
    return maps


def _combine(results):
    y = np.zeros((B, S, DIM), np.float32)
    for c in range(NCORES):
        y[c // HG] += results[c]["y"]
    return y


def kernel(x, mask, wq, wk, wv, wo, **run_kwargs):
    from concourse.bass_utils import run_bass_kernel_spmd

    nc = _get_nc()
    res = run_bass_kernel_spmd(
        nc, _in_maps(x, mask, wq, wk, wv, wo), core_ids=list(range(NCORES)),
        **run_kwargs,
    )
    out = _combine(res.results)
    if run_kwargs:
        _CACHE["last_result"] = res
    return out


# revision 7
# speedup vs baseline: 1.2821x; 1.2821x over previous
"""Multi-head causal attention (B=2, S=2048, D=1024, H=16, hd=64) on 8 trn2
NeuronCores.

Sharding: core c -> batch b=c//4, head-group g=c%4 (4 heads = 256 contiguous
model dims). Each core computes q/k/v projections for its head group from the
full (transposed) batch-b input, runs causal attention for its 4 heads, and
applies its slice of the output projection, producing a partial [2048, 1024]
output. The host sums the 4 partials per batch.

Matmul operands are bf16 (PE 1 cycle/row; fp32r measured ~3 cycles/row on HW);
accumulation is fp32 in PSUM. End-to-end rel err vs the fp32 reference is
~4e-3 (numpy-verified).

Scores are computed transposed (S^T[j,i] = k^T.T @ q^T) so the softmax
denominator comes free from the AV matmul via a ones-column appended to V,
and no transposes are needed anywhere. Causality is structural: only j<=i
blocks are computed; the 128x128 diagonal blocks get the (scaled) mask added.
exp() skips max-subtraction (scores are ~N(0,1); fp32 exp is safe and masked
entries underflow to exactly 0, matching the reference softmax up to
rounding).
"""

import sys

for p in ("/opt/trn_rl_repo", "/root/.axon_site/_ro/trn_rl_repo"):
    if p not in sys.path:
        sys.path.insert(0, p)

import ml_dtypes
import numpy as np

B, S, DIM, H, HD = 2, 2048, 1024, 16, 64
NCORES = 8
HG = 4  # heads per core
OG = HG * HD  # 256 output dims per core
NB = S // 512  # 4 i-blocks of 512
NJ = S // 128  # 16 j-tiles of 128

_CACHE = {}


def _build():
    import concourse.tile as tile
    from concourse import bacc, mybir

    f32 = mybir.dt.float32
    bf16 = mybir.dt.bfloat16
    Exp = mybir.ActivationFunctionType.Exp

    nc = bacc.Bacc("TRN2", target_bir_lowering=False, debug=False, num_devices=NCORES)

    xT = nc.dram_tensor("xT", [DIM, S], bf16, kind="ExternalInput")
    wqT = nc.dram_tensor("wqT", [DIM, OG], bf16, kind="ExternalInput")
    wkT = nc.dram_tensor("wkT", [DIM, OG], bf16, kind="ExternalInput")
    wvT = nc.dram_tensor("wvT", [DIM, OG], bf16, kind="ExternalInput")
    woT = nc.dram_tensor("woT", [OG, DIM], bf16, kind="ExternalInput")
    cmask8 = nc.dram_tensor("cmask8", [128, 128], f32, kind="ExternalInput")
    onesd = nc.dram_tensor("onesd", [128, 64], bf16, kind="ExternalInput")
    y = nc.dram_tensor("y", [S, DIM], f32, kind="ExternalOutput")

    xT_r = xT.ap().rearrange("(t p) s -> t p s", p=128)  # [8,128,2048]
    wqT_r = wqT.ap().rearrange("(t p) o -> t p o", p=128)  # [8,128,256]
    wkT_r = wkT.ap().rearrange("(t p) o -> t p o", p=128)
    wvT_r = wvT.ap().rearrange("(t p) o -> t p o", p=128)
    woT_r = woT.ap().rearrange("(t p) e -> t p e", p=128)  # [2,128,1024]
    y_r = y.ap().rearrange("(t p) e -> t p e", p=128)  # [16,128,1024]

    with tile.TileContext(nc) as tc:
        with (
            tc.tile_pool(name="persist", bufs=1) as pp,
            tc.tile_pool(name="work", bufs=4) as wp,
            tc.tile_pool(name="psum", bufs=4, space="PSUM") as ps,
        ):
            # ---- persistent SBUF tiles -------------------------------------
            xt = [pp.tile([128, S], bf16, tag=f"xt{i}", name=f"xt{i}") for i in range(8)]
            wqt = [pp.tile([128, OG], bf16, tag=f"wq{i}", name=f"wq{i}") for i in range(8)]
            wkt = [pp.tile([128, OG], bf16, tag=f"wk{i}", name=f"wk{i}") for i in range(8)]
            wvt = [pp.tile([128, OG], bf16, tag=f"wv{i}", name=f"wv{i}") for i in range(8)]
            wot = [pp.tile([128, DIM], bf16, tag=f"wo{i}", name=f"wo{i}") for i in range(2)]
            cm = pp.tile([128, 128], f32, tag="cm")
            qT = [pp.tile([128, S], bf16, tag=f"qT{i}", name=f"qT{i}") for i in range(2)]
            kT = [pp.tile([128, S], bf16, tag=f"kT{i}", name=f"kT{i}") for i in range(2)]
            vv = [pp.tile([128, HG, HD + 1], bf16, tag=f"vv{i}", name=f"vv{i}") for i in range(NJ)]
            zT = [pp.tile([128, S], bf16, tag=f"zT{i}", name=f"zT{i}") for i in range(2)]
            ones1 = pp.tile([1, 64], bf16, tag="ones1")

            for i in range(8):
                nc.sync.dma_start(out=xt[i], in_=xT_r[i])
                nc.sync.dma_start(out=wkt[i], in_=wkT_r[i])
                nc.sync.dma_start(out=wqt[i], in_=wqT_r[i])
                nc.sync.dma_start(out=wvt[i], in_=wvT_r[i])
            for i in range(2):
                nc.sync.dma_start(out=wot[i], in_=woT_r[i])
            nc.sync.dma_start(out=cm, in_=cmask8.ap())
            nc.sync.dma_start(out=ones1, in_=onesd.ap()[0:1, :])

            # ---- q/k/v projections ----------------------------------------
            # qT/kT: [o 128-chunk, s 512-block] accumulated over 8 e-tiles.
            for m in range(2):
                for n in range(NB):
                    for dst, wt in ((kT, wkt), (qT, wqt)):
                        acc2 = ps.tile([128, 1024], f32, tag="s2", bufs=3)
                        acc = acc2[:, 0:512]
                        for e in range(8):
                            nc.tensor.matmul(
                                acc,
                                wt[e][:, m * 128 : (m + 1) * 128],
                                xt[e][:, n * 512 : (n + 1) * 512],
                                start=(e == 0),
                                stop=(e == 7),
                            )
                        nc.scalar.copy(dst[m][:, n * 512 : (n + 1) * 512], acc)
            # v: natural layout [s 128-chunk, o], with a ones column per head.
            for s in range(NJ):
                acc2 = ps.tile([128, 1024], f32, tag="s2", bufs=3)
                acc = acc2[:, 0:512]
                for e in range(8):
                    nc.tensor.matmul(
                        acc[:, 0:OG],
                        xt[e][:, s * 128 : (s + 1) * 128],
                        wvt[e],
                        start=(e == 0),
                        stop=(e == 7),
                    )
                nc.vector.tensor_copy(
                    vv[s][:, :, 0:HD],
                    acc[:, 0:OG].rearrange("p (h d) -> p h d", h=HG),
                )
                nc.sync.dma_start(
                    out=vv[s][:, :, HD : HD + 1],
                    in_=onesd.ap()[:, 0:HG].rearrange("p (h o) -> p h o", o=1),
                )

            # ---- attention per head ---------------------------------------
            for h in range(HG):
                m, po = divmod(h, 2)
                po *= 64
                for ib in range(NB):
                    psz = ps.tile([65, 512], f32, tag="z", bufs=2)
                    nplain = 4 * ib
                    njb = nplain + 4
                    jb = 0
                    # paired plain blocks: two S matmuls into one 2-bank psum
                    # tile, one exp over [128, 1024], two AV matmuls.
                    while jb + 1 < nplain:
                        pss = ps.tile([128, 1024], f32, tag="s2", bufs=3)
                        for u in range(2):
                            nc.tensor.matmul(
                                pss[:, u * 512 : (u + 1) * 512],
                                kT[m][po : po + 64, (jb + u) * 128 : (jb + u + 1) * 128],
                                qT[m][po : po + 64, ib * 512 : (ib + 1) * 512],
                                start=True,
                                stop=True,
                            )
                        ex = wp.tile([128, 1024], bf16, tag="ex")
                        nc.scalar.activation(ex, pss, Exp, scale=0.125)
                        for u in range(2):
                            nc.tensor.matmul(
                                psz,
                                vv[jb + u][:, h, :],
                                ex[:, u * 512 : (u + 1) * 512],
                                start=(jb + u == 0),
                                stop=False,
                            )
                        jb += 2
                    # diagonal band blocks
                    for t in range(4):
                        jb = nplain + t
                        off = 128 * t
                        last = t == 3
                        ps2 = ps.tile([128, 1024], f32, tag="s2", bufs=3)
                        pss = ps2[:, 0:512]
                        nc.tensor.matmul(
                            pss[:, off:512],
                            kT[m][po : po + 64, jb * 128 : (jb + 1) * 128],
                            qT[m][po : po + 64, ib * 512 + off : (ib + 1) * 512],
                            start=True,
                            stop=True,
                        )
                        ex = wp.tile([128, 512], bf16, tag="exb")
                        sd = wp.tile([128, 128], f32, tag="sd", bufs=3)
                        nc.vector.tensor_add(sd, pss[:, off : off + 128], cm)
                        nc.scalar.activation(
                            ex[:, off : off + 128], sd, Exp, scale=0.125
                        )
                        if t < 3:
                            nc.scalar.activation(
                                ex[:, off + 128 : 512],
                                pss[:, off + 128 : 512],
                                Exp,
                                scale=0.125,
                            )
                        nc.tensor.matmul(
                            psz[:, off:512],
                            vv[jb][:, h, :],
                            ex[:, off:512],
                            start=(jb == 0),
                            stop=last,
                        )
                    # normalize: psz row 64 holds the softmax denominator.
                    dn = wp.tile([1, 512], bf16, tag="dn", bufs=2)
                    nc.vector.tensor_copy(dn, psz[64:65, :])
                    psb2 = ps.tile([128, 1024], f32, tag="s2", bufs=3)
                    psb = psb2[:, 0:512]
                    nc.tensor.matmul(
                        psb[0:64, 0:512], ones1, dn, start=True, stop=True
                    )
                    rc = wp.tile([64, 512], f32, tag="rc", bufs=2)
                    nc.vector.reciprocal_approx_fast(rc, psb[0:64, :])
                    nc.vector.tensor_mul(
                        zT[m][po : po + 64, ib * 512 : (ib + 1) * 512],
                        psz[0:64, :],
                        rc,
                    )

            # ---- output projection ----------------------------------------
            for s in range(NJ):
                ysb = wp.tile([128, DIM], f32, tag="ysb", bufs=3, name="ysb")
                for n2 in range(2):
                    psy2 = ps.tile([128, 1024], f32, tag="s2", bufs=3)
                    psy = psy2[:, 0:512]
                    for kk in range(2):
                        nc.tensor.matmul(
                            psy,
                            zT[kk][:, s * 128 : (s + 1) * 128],
                            wot[kk][:, n2 * 512 : (n2 + 1) * 512],
                            start=(kk == 0),
                            stop=(kk == 1),
                        )
                    if n2 == 0:
                        nc.scalar.copy(ysb[:, 0:512], psy)
                    else:
                        nc.vector.tensor_copy(ysb[:, 512:1024], psy)
                nc.sync.dma_start(out=y_r[s], in_=ysb)

    nc.compile()
    return nc


def _get_nc():
    if "nc" not in _CACHE:
        _CACHE["nc"] = _build()
    return _CACHE["nc"]


def _in_maps(x, mask, wq, wk, wv, wo):
    bf = ml_dtypes.bfloat16
    cm8 = np.ascontiguousarray(8.0 * np.asarray(mask)[0, 0, :128, :128].T, np.float32)
    maps = []
    for c in range(NCORES):
        b, g = divmod(c, HG)
        sl = slice(OG * g, OG * (g + 1))
        maps.append(
            {
                "xT": np.ascontiguousarray(np.asarray(x)[b].T).astype(bf),
                "wqT": np.ascontiguousarray(np.asarray(wq)[sl, :].T).astype(bf),
                "wkT": np.ascontiguousarray(np.asarray(wk)[sl, :].T).astype(bf),
                "wvT": np.ascontiguousarray(np.asarray(wv)[sl, :].T).astype(bf),
                "woT": np.ascontiguousarray(np.asarray(wo)[:, sl].T).astype(bf),
                "cmask8": cm8,
                "onesd": np.ones((128, 64), bf),
            }
        )
    return maps


def _combine(results):
    y = np.zeros((B, S, DIM), np.float32)
    for c in range(NCORES):
        y[c // HG] += results[c]["y"]
    return y


def kernel(x, mask, wq, wk, wv, wo, **run_kwargs):
    from concourse.bass_utils import run_bass_kernel_spmd

    nc = _get_nc()
    res = run_bass_kernel_spmd(
        nc, _in_maps(x, mask, wq, wk, wv, wo), core_ids=list(range(NCORES)),
        **run_kwargs,
    )
    out = _combine(res.results)
    if run_kwargs:
        _CACHE["last_result"] = res
    return out


# revision 8
# speedup vs baseline: 1.2822x; 1.0001x over previous
"""Multi-head causal attention (B=2, S=2048, D=1024, H=16, hd=64) on 8 trn2
NeuronCores.

Sharding: core c -> batch b=c//4, head-group g=c%4 (4 heads = 256 contiguous
model dims). Each core computes q/k/v projections for its head group from the
full (transposed) batch-b input, runs causal attention for its 4 heads, and
applies its slice of the output projection, producing a partial [2048, 1024]
output. The host sums the 4 partials per batch.

Matmul operands are bf16 (PE 1 cycle/row; fp32r measured ~3 cycles/row on HW);
accumulation is fp32 in PSUM. End-to-end rel err vs the fp32 reference is
~4e-3 (numpy-verified).

Scores are computed transposed (S^T[j,i] = k^T.T @ q^T) so the softmax
denominator comes free from the AV matmul via a ones-column appended to V,
and no transposes are needed anywhere. Causality is structural: only j<=i
blocks are computed; the 128x128 diagonal blocks get the (scaled) mask added.
exp() skips max-subtraction (scores are ~N(0,1); fp32 exp is safe and masked
entries underflow to exactly 0, matching the reference softmax up to
rounding).
"""

import sys

for p in ("/opt/trn_rl_repo", "/root/.axon_site/_ro/trn_rl_repo"):
    if p not in sys.path:
        sys.path.insert(0, p)

import ml_dtypes
import numpy as np

B, S, DIM, H, HD = 2, 2048, 1024, 16, 64
NCORES = 8
HG = 4  # heads per core
OG = HG * HD  # 256 output dims per core
NB = S // 512  # 4 i-blocks of 512
NJ = S // 128  # 16 j-tiles of 128

_CACHE = {}


def _build():
    import concourse.tile as tile
    from concourse import bacc, mybir

    f32 = mybir.dt.float32
    bf16 = mybir.dt.bfloat16
    Exp = mybir.ActivationFunctionType.Exp

    nc = bacc.Bacc("TRN2", target_bir_lowering=False, debug=False, num_devices=NCORES)

    xT = nc.dram_tensor("xT", [DIM, S], bf16, kind="ExternalInput")
    wqT = nc.dram_tensor("wqT", [DIM, OG], bf16, kind="ExternalInput")
    wkT = nc.dram_tensor("wkT", [DIM, OG], bf16, kind="ExternalInput")
    wvT = nc.dram_tensor("wvT", [DIM, OG], bf16, kind="ExternalInput")
    woT = nc.dram_tensor("woT", [OG, DIM], bf16, kind="ExternalInput")
    cmask8 = nc.dram_tensor("cmask8", [128, 128], f32, kind="ExternalInput")
    onesd = nc.dram_tensor("onesd", [128, 64], bf16, kind="ExternalInput")
    y = nc.dram_tensor("y", [S, DIM], f32, kind="ExternalOutput")

    xT_r = xT.ap().rearrange("(t p) s -> t p s", p=128)  # [8,128,2048]
    wqT_r = wqT.ap().rearrange("(t p) o -> t p o", p=128)  # [8,128,256]
    wkT_r = wkT.ap().rearrange("(t p) o -> t p o", p=128)
    wvT_r = wvT.ap().rearrange("(t p) o -> t p o", p=128)
    woT_r = woT.ap().rearrange("(t p) e -> t p e", p=128)  # [2,128,1024]
    y_r = y.ap().rearrange("(t p) e -> t p e", p=128)  # [16,128,1024]

    with tile.TileContext(nc) as tc:
        with (
            tc.tile_pool(name="persist", bufs=1) as pp,
            tc.tile_pool(name="work", bufs=4) as wp,
            tc.tile_pool(name="psum", bufs=4, space="PSUM") as ps,
        ):
            # ---- persistent SBUF tiles -------------------------------------
            xt = [pp.tile([128, S], bf16, tag=f"xt{i}", name=f"xt{i}") for i in range(8)]
            wqt = [pp.tile([128, OG], bf16, tag=f"wq{i}", name=f"wq{i}") for i in range(8)]
            wkt = [pp.tile([128, OG], bf16, tag=f"wk{i}", name=f"wk{i}") for i in range(8)]
            wvt = [pp.tile([128, OG], bf16, tag=f"wv{i}", name=f"wv{i}") for i in range(8)]
            wot = [pp.tile([128, DIM], bf16, tag=f"wo{i}", name=f"wo{i}") for i in range(2)]
            cm = pp.tile([128, 128], f32, tag="cm")
            qT = [pp.tile([128, S], bf16, tag=f"qT{i}", name=f"qT{i}") for i in range(2)]
            kT = [pp.tile([128, S], bf16, tag=f"kT{i}", name=f"kT{i}") for i in range(2)]
            vv = [pp.tile([128, HG, HD + 1], bf16, tag=f"vv{i}", name=f"vv{i}") for i in range(NJ)]
            zT = [pp.tile([128, S], bf16, tag=f"zT{i}", name=f"zT{i}") for i in range(2)]
            ones1 = pp.tile([1, 64], bf16, tag="ones1")

            for i in range(8):
                nc.sync.dma_start(out=xt[i], in_=xT_r[i])
                nc.sync.dma_start(out=wkt[i], in_=wkT_r[i])
                nc.sync.dma_start(out=wqt[i], in_=wqT_r[i])
                nc.sync.dma_start(out=wvt[i], in_=wvT_r[i])
            for i in range(2):
                nc.sync.dma_start(out=wot[i], in_=woT_r[i])
            nc.sync.dma_start(out=cm, in_=cmask8.ap())
            nc.sync.dma_start(out=ones1, in_=onesd.ap()[0:1, :])

            # ---- q/k/v projections ----------------------------------------
            # qT/kT: [o 128-chunk, s 512-block] accumulated over 8 e-tiles.
            for m in range(2):
                for n in range(NB):
                    for dst, wt in ((kT, wkt), (qT, wqt)):
                        acc2 = ps.tile([128, 1024], f32, tag="s2", bufs=3)
                        acc = acc2[:, 0:512]
                        for e in range(8):
                            nc.tensor.matmul(
                                acc,
                                wt[e][:, m * 128 : (m + 1) * 128],
                                xt[e][:, n * 512 : (n + 1) * 512],
                                start=(e == 0),
                                stop=(e == 7),
                            )
                        nc.scalar.copy(dst[m][:, n * 512 : (n + 1) * 512], acc)
            # v: natural layout [s 128-chunk, o], with a ones column per head.
            for s in range(NJ):
                acc2 = ps.tile([128, 1024], f32, tag="s2", bufs=3)
                acc = acc2[:, 0:512]
                for e in range(8):
                    nc.tensor.matmul(
                        acc[:, 0:OG],
                        xt[e][:, s * 128 : (s + 1) * 128],
                        wvt[e],
                        start=(e == 0),
                        stop=(e == 7),
                    )
                nc.vector.tensor_copy(
                    vv[s][:, :, 0:HD],
                    acc[:, 0:OG].rearrange("p (h d) -> p h d", h=HG),
                )
                nc.sync.dma_start(
                    out=vv[s][:, :, HD : HD + 1],
                    in_=onesd.ap()[:, 0:HG].rearrange("p (h o) -> p h o", o=1),
                )

            # ---- attention per head ---------------------------------------
            for h in range(HG):
                m, po = divmod(h, 2)
                po *= 64
                for ib in range(NB):
                    psz = ps.tile([65, 512], f32, tag="z", bufs=2)
                    nplain = 4 * ib
                    njb = nplain + 4
                    jb = 0
                    # paired plain blocks: two S matmuls into one 2-bank psum
                    # tile, one exp over [128, 1024], two AV matmuls.
                    while jb + 1 < nplain:
                        pss = ps.tile([128, 1024], f32, tag="s2", bufs=3)
                        for u in range(2):
                            nc.tensor.matmul(
                                pss[:, u * 512 : (u + 1) * 512],
                                kT[m][po : po + 64, (jb + u) * 128 : (jb + u + 1) * 128],
                                qT[m][po : po + 64, ib * 512 : (ib + 1) * 512],
                                start=True,
                                stop=True,
                            )
                        ex = wp.tile([128, 1024], bf16, tag="ex")
                        nc.scalar.activation(ex, pss, Exp, scale=0.125)
                        for u in range(2):
                            nc.tensor.matmul(
                                psz,
                                vv[jb + u][:, h, :],
                                ex[:, u * 512 : (u + 1) * 512],
                                start=(jb + u == 0),
                                stop=False,
                            )
                        jb += 2
                    # diagonal band blocks
                    for t in range(4):
                        jb = nplain + t
                        off = 128 * t
                        last = t == 3
                        ps2 = ps.tile([128, 1024], f32, tag="s2", bufs=3)
                        pss = ps2[:, 0:512]
                        nc.tensor.matmul(
                            pss[:, off:512],
                            kT[m][po : po + 64, jb * 128 : (jb + 1) * 128],
                            qT[m][po : po + 64, ib * 512 + off : (ib + 1) * 512],
                            start=True,
                            stop=True,
                        )
                        ex = wp.tile([128, 512], bf16, tag="exb")
                        nc.vector.tensor_add(
                            pss[:, off : off + 128], pss[:, off : off + 128], cm
                        )
                        nc.scalar.activation(
                            ex[:, off:512], pss[:, off:512], Exp, scale=0.125
                        )
                        nc.tensor.matmul(
                            psz[:, off:512],
                            vv[jb][:, h, :],
                            ex[:, off:512],
                            start=(jb == 0),
                            stop=last,
                        )
                    # normalize: psz row 64 holds the softmax denominator.
                    dn = wp.tile([1, 512], bf16, tag="dn", bufs=2)
                    nc.vector.tensor_copy(dn, psz[64:65, :])
                    psb2 = ps.tile([128, 1024], f32, tag="s2", bufs=3)
                    psb = psb2[:, 0:512]
                    nc.tensor.matmul(
                        psb[0:64, 0:512], ones1, dn, start=True, stop=True
                    )
                    rc = wp.tile([64, 512], f32, tag="rc", bufs=2)
                    nc.vector.reciprocal_approx_fast(rc, psb[0:64, :])
                    nc.vector.tensor_mul(
                        zT[m][po : po + 64, ib * 512 : (ib + 1) * 512],
                        psz[0:64, :],
                        rc,
                    )

            # ---- output projection ----------------------------------------
            for s in range(NJ):
                ysb = wp.tile([128, DIM], f32, tag="ysb", bufs=3, name="ysb")
                for n2 in range(2):
                    psy2 = ps.tile([128, 1024], f32, tag="s2", bufs=3)
                    psy = psy2[:, 0:512]
                    for kk in range(2):
                        nc.tensor.matmul(
                            psy,
                            zT[kk][:, s * 128 : (s + 1) * 128],
                            wot[kk][:, n2 * 512 : (n2 + 1) * 512],
                            start=(kk == 0),
                            stop=(kk == 1),
                        )
                    if n2 == 0:
                        nc.scalar.copy(ysb[:, 0:512], psy)
                    else:
                        nc.vector.tensor_copy(ysb[:, 512:1024], psy)
                nc.sync.dma_start(out=y_r[s], in_=ysb)

    nc.compile()
    return nc


def _get_nc():
    if "nc" not in _CACHE:
        _CACHE["nc"] = _build()
    return _CACHE["nc"]


def _in_maps(x, mask, wq, wk, wv, wo):
    bf = ml_dtypes.bfloat16
    cm8 = np.ascontiguousarray(8.0 * np.asarray(mask)[0, 0, :128, :128].T, np.float32)
    maps = []
    for c in range(NCORES):
        b, g = divmod(c, HG)
        sl = slice(OG * g, OG * (g + 1))
        maps.append(
            {
                "xT": np.ascontiguousarray(np.asarray(x)[b].T).astype(bf),
                "wqT": np.ascontiguousarray(np.asarray(wq)[sl, :].T).astype(bf),
                "wkT": np.ascontiguousarray(np.asarray(wk)[sl, :].T).astype(bf),
                "wvT": np.ascontiguousarray(np.asarray(wv)[sl, :].T).astype(bf),
                "woT": np.ascontiguousarray(np.asarray(wo)[:, sl].T).astype(bf),
                "cmask8": cm8,
                "onesd": np.ones((128, 64), bf),
            }
        )
    return maps


def _combine(results):
    y = np.zeros((B, S, DIM), np.float32)
    for c in range(NCORES):
        y[c // HG] += results[c]["y"]
    return y


def kernel(x, mask, wq, wk, wv, wo, **run_kwargs):
    from concourse.bass_utils import run_bass_kernel_spmd

    nc = _get_nc()
    res = run_bass_kernel_spmd(
        nc, _in_maps(x, mask, wq, wk, wv, wo), core_ids=list(range(NCORES)),
        **run_kwargs,
    )
    out = _combine(res.results)
    if run_kwargs:
        _CACHE["last_result"] = res
    return out
